# revision 5
# baseline (speedup 1.0000x reference)
"""Trainium2 Bass kernel for nn_RNNModel loss (RNN scan + contrastive sample loss).

Strategy (8 cores, data-parallel):
  - Project token table P' = emb @ W_ih.T + (b_ih + b_hh), sharded 4000 rows/core,
    AllGather -> full projected table (bf16). Sample "matmuls" become row gathers.
  - Wx for the scan = same projection of the 8192 data tokens, sharded 1024
    rows/core + AllGather (computed directly so the scan can start early).
  - RNN scan (128 steps, [64,1024] hidden) replicated on every core: 18 bf16
    matmuls/step accumulating Wx (identity-matmul) + U@h in PSUM, tanh on ACT,
    h transposed for the next step via DMA-transpose. Positive pairwise term
    accumulated in-scan. h trajectory stored to DRAM (bf16).
  - Negative block position-sharded: core c handles positions [1024c, 1024c+1024)
    for all 10 samples: gather prev rows, hiddens_U matmul, gather projected
    sample rows, add + tanh + squared-distance (ACT Square w/ accumulate),
    clip/exp/log reduce -> scalar partial.
  - Host sums per-core partials (pos from core 0; neg from all cores).
"""

import numpy as np
import ml_dtypes
from contextlib import ExitStack

V, H, S, B, NS, NC = 32000, 1024, 128, 64, 10, 8
N = S * B            # 8192 positions
VSH = V // NC        # 4000 table rows per core
PSH = N // NC        # 1024 positions per core
TEMP, CLIP_DIST, EPS = 65.0, 0.01, 1e-6

_CACHE = {}


def _build():
    import concourse.bass as bass
    import concourse.tile as tile
    from concourse import bacc, mybir
    from concourse.masks import make_identity

    f32 = mybir.dt.float32
    bf16 = mybir.dt.bfloat16
    i32 = mybir.dt.int32
    AF = mybir.ActivationFunctionType
    OP = mybir.AluOpType

    nc = bacc.Bacc("TRN2", target_bir_lowering=False, debug=False, num_devices=NC)

    # ---- I/O ----
    emb = nc.dram_tensor("emb", [V, H], f32, kind="ExternalInput")
    wihT = nc.dram_tensor("wihT", [H, H], bf16, kind="ExternalInput")
    whhT = nc.dram_tensor("whhT", [H, H], bf16, kind="ExternalInput")
    bias2 = nc.dram_tensor("bias2", [1, H], bf16, kind="ExternalInput")
    wx_idx = nc.dram_tensor("wx_idx", [PSH, 1], i32, kind="ExternalInput")
    ps_idx = nc.dram_tensor("ps_idx", [VSH, 1], i32, kind="ExternalInput")
    samp_idx = nc.dram_tensor("samp_idx", [NS * PSH, 1], i32, kind="ExternalInput")
    prev_idx = nc.dram_tensor("prev_idx", [PSH, 1], i32, kind="ExternalInput")
    pos_out = nc.dram_tensor("pos_out", [1, 1], f32, kind="ExternalOutput")
    neg_out = nc.dram_tensor("neg_out", [1, 1], f32, kind="ExternalOutput")

    # ---- internal DRAM ----
    wx_sh = nc.dram_tensor("wx_sh", [PSH, H], bf16)
    wx_all = nc.dram_tensor("wx_all", [N, H], bf16, addr_space="Shared")
    p_sh = nc.dram_tensor("p_sh", [VSH, H], bf16)
    p_all = nc.dram_tensor("p_all", [V, H], bf16, addr_space="Shared")
    raw = nc.dram_tensor("raw", [N, H], bf16)

    groups = [list(range(NC))]

    with tile.TileContext(nc) as tc, ExitStack() as ctx:
        const = ctx.enter_context(tc.tile_pool(name="const", bufs=1))
        io = ctx.enter_context(tc.tile_pool(name="io", bufs=4))
        wk = ctx.enter_context(tc.tile_pool(name="wk", bufs=3))
        hp = ctx.enter_context(tc.tile_pool(name="hp", bufs=2))
        pp_scan = ctx.enter_context(tc.tile_pool(name="pp_scan", bufs=2, space="PSUM"))
        pp_big = ctx.enter_context(tc.tile_pool(name="pp_big", bufs=2, space="PSUM"))

        # ---- constants / weights in SBUF ----
        wihT_sb = const.tile([128, 8 * H], bf16)
        whhT_sb = const.tile([128, 8 * H], bf16)
        for kt in range(8):
            nc.sync.dma_start(wihT_sb[:, kt * H:(kt + 1) * H], wihT[kt * 128:(kt + 1) * 128, :])
            nc.sync.dma_start(whhT_sb[:, kt * H:(kt + 1) * H], whhT[kt * 128:(kt + 1) * 128, :])
        bias2_sb = const.tile([1, H], bf16)
        nc.sync.dma_start(bias2_sb[:], bias2[:, :])
        ones1 = const.tile([1, 128], bf16)
        nc.vector.memset(ones1[:], 1.0)
        I64 = const.tile([64, 64], bf16)
        make_identity(nc, I64[:])
        ones64f = const.tile([64, 1], f32)
        nc.vector.memset(ones64f[:], 1.0)
        ones128f = const.tile([128, 1], f32)
        nc.vector.memset(ones128f[:], 1.0)
        pos_acc = const.tile([64, 1], f32)
        nc.vector.memset(pos_acc[:], 0.0)
        eps64 = const.tile([64, 1], f32)
        nc.vector.memset(eps64[:], EPS)
        eps128 = const.tile([128, 1], f32)
        nc.vector.memset(eps128[:], EPS)
        negmat = const.tile([128, 8], f32)

        # ---- projection tile: rows of emb -> rows of (e @ W_ih.T + bias2), bf16 -> dst
        def proj_tile(idx_ap, dst_ap, it, rows):
            idx_t = io.tile([128, 1], i32, tag="idx")
            nc.sync.dma_start(idx_t[:rows], idx_ap[it * 128: it * 128 + rows, :])
            ew = wk.tile([128, H], f32, tag="ew")
            nc.gpsimd.indirect_dma_start(
                out=ew[:rows], out_offset=None, in_=emb[:, :],
                in_offset=bass.IndirectOffsetOnAxis(ap=idx_t[:rows, :1], axis=0))
            ewb = wk.tile([128, H], bf16, tag="ewb")
            nc.vector.tensor_copy(ewb[:rows], ew[:rows])
            eT = wk.tile([128, 8 * 128], bf16, tag="eT")
            nc.sync.dma_start_transpose(
                out=eT[:].rearrange("p (k b) -> p k b", b=128)[:, :, :rows],
                in_=ewb[:rows, :])
            ps = pp_big.tile([128, H], f32, tag="proj_ps")
            for half in range(2):
                sl = slice(half * 512, (half + 1) * 512)
                nc.tensor.matmul(ps[:rows, sl], lhsT=ones1[:1, :rows],
                                 rhs=bias2_sb[:1, sl], start=True, stop=False)
                for k in range(8):
                    nc.tensor.matmul(
                        ps[:rows, sl],
                        lhsT=eT[:, k * 128: k * 128 + rows],
                        rhs=wihT_sb[:, k * H + half * 512: k * H + (half + 1) * 512],
                        start=False, stop=(k == 7))
            ob = wk.tile([128, H], bf16, tag="ob")
            nc.vector.tensor_copy(ob[:rows], ps[:rows])
            nc.sync.dma_start(dst_ap[it * 128: it * 128 + rows, :], ob[:rows])

        # ---- Wx shard + AllGather ----
        for it in range(PSH // 128):
            proj_tile(wx_idx, wx_sh, it, 128)
        nc.gpsimd.collective_compute(
            "AllGather", mybir.AluOpType.bypass, replica_groups=groups,
            ins=[wx_sh.ap().opt()], outs=[wx_all.ap().opt()])

        # ---- P' shard tiles (interleaved into scan below) ----
        n_ptiles = (VSH + 127) // 128  # 32 (last tile has 32 rows)

        def p_tile(i):
            rows = min(128, VSH - i * 128)
            proj_tile(ps_idx, p_sh, i, rows)

        # ---- scan init ----
        h_prev = hp.tile([64, H], bf16, tag="h")
        nc.vector.memset(h_prev[:], 0.0)
        hT_prev = hp.tile([128, 8 * 64], bf16, tag="hT")
        nc.vector.memset(hT_prev[:], 0.0)
        nc.sync.dma_start(raw[0:64, :], h_prev[:])

        # ---- scan ----
        for t in range(1, S + 1):
            # interleave projected-table tiles into the first 64 steps
            if t % 2 == 1 and (t - 1) // 2 < n_ptiles:
                p_tile((t - 1) // 2)
            wx_t = io.tile([64, H], bf16, tag="wx")
            nc.sync.dma_start(wx_t[:], wx_all[(t - 1) * 64: t * 64, :])
            ps = pp_scan.tile([64, H], f32, tag="scan_ps")
            for half in range(2):
                sl = slice(half * 512, (half + 1) * 512)
                nc.tensor.matmul(ps[:, sl], lhsT=I64[:], rhs=wx_t[:, sl],
                                 start=True, stop=False)
                for k in range(8):
                    nc.tensor.matmul(
                        ps[:, sl],
                        lhsT=hT_prev[:, k * 64:(k + 1) * 64],
                        rhs=whhT_sb[:, k * H + half * 512: k * H + (half + 1) * 512],
                        start=False, stop=(k == 7))
            h_cur = hp.tile([64, H], bf16, tag="h")
            nc.scalar.activation(h_cur[:, 0:512], ps[:, 0:512], AF.Tanh)
            nc.scalar.activation(h_cur[:, 512:1024], ps[:, 512:1024], AF.Tanh)
            # positive term: (h_{t-1} - h_t + eps)^2 summed
            d = wk.tile([64, H], bf16, tag="d")
            nc.vector.tensor_tensor(out=d[:], in0=h_prev[:], in1=h_cur[:], op=OP.subtract)
            sq = wk.tile([64, H], bf16, tag="sq")
            posc = wk.tile([64, 1], f32, tag="posc")
            nc.scalar.activation(sq[:], d[:], AF.Square, bias=eps64[:], scale=1.0,
                                 accum_out=posc[:])
            nc.vector.tensor_tensor(out=pos_acc[:], in0=pos_acc[:], in1=posc[:], op=OP.add)
            if t < S:
                nc.sync.dma_start(raw[t * 64:(t + 1) * 64, :], h_cur[:])
                hT_cur = hp.tile([128, 8 * 64], bf16, tag="hT")
                nc.sync.dma_start_transpose(
                    out=hT_cur[:, 0:256].rearrange("p (k b) -> p k b", b=64),
                    in_=h_cur[:, 0:512])
                nc.sync.dma_start_transpose(
                    out=hT_cur[:, 256:512].rearrange("p (k b) -> p k b", b=64),
                    in_=h_cur[:, 512:1024])
                hT_prev = hT_cur
            h_prev = h_cur

        # AllGather the projected table (shards all written during scan)
        nc.gpsimd.collective_compute(
            "AllGather", mybir.AluOpType.bypass, replica_groups=groups,
            ins=[p_sh.ap().opt()], outs=[p_all.ap().opt()])

        # ---- negative block: 8 position-tiles x 10 samples ----
        for pt in range(8):
            pidx_t = io.tile([128, 1], i32, tag="idx")
            nc.sync.dma_start(pidx_t[:], prev_idx[pt * 128:(pt + 1) * 128, :])
            prev_t = wk.tile([128, H], bf16, tag="prev")
            nc.gpsimd.indirect_dma_start(
                out=prev_t[:], out_offset=None, in_=raw[:, :],
                in_offset=bass.IndirectOffsetOnAxis(ap=pidx_t[:, :1], axis=0))
            prevT = wk.tile([128, 8 * 128], bf16, tag="prevT")
            nc.sync.dma_start_transpose(
                out=prevT[:].rearrange("p (k b) -> p k b", b=128),
                in_=prev_t[:])
            ps = pp_big.tile([128, H], f32, tag="proj_ps")
            for half in range(2):
                sl = slice(half * 512, (half + 1) * 512)
                for k in range(8):
                    nc.tensor.matmul(
                        ps[:, sl],
                        lhsT=prevT[:, k * 128:(k + 1) * 128],
                        rhs=whhT_sb[:, k * H + half * 512: k * H + (half + 1) * 512],
                        start=(k == 0), stop=(k == 7))
            hU = wk.tile([128, H], bf16, tag="hU")
            nc.vector.tensor_copy(hU[:], ps[:])
            dmat = wk.tile([128, NS], f32, tag="dmat")
            for s in range(NS):
                sidx_t = io.tile([128, 1], i32, tag="idx")
                nc.sync.dma_start(sidx_t[:], samp_idx[(s * 8 + pt) * 128:(s * 8 + pt + 1) * 128, :])
                spw = wk.tile([128, H], bf16, tag="spw")
                nc.gpsimd.indirect_dma_start(
                    out=spw[:], out_offset=None, in_=p_all[:, :],
                    in_offset=bass.IndirectOffsetOnAxis(ap=sidx_t[:, :1], axis=0))
                pre = wk.tile([128, H], bf16, tag="pre")
                nc.vector.tensor_tensor(out=pre[:], in0=spw[:], in1=hU[:], op=OP.add)
                outt = wk.tile([128, H], bf16, tag="outt")
                nc.scalar.activation(outt[:], pre[:], AF.Tanh)
                dneg = wk.tile([128, H], bf16, tag="dneg")
                nc.vector.tensor_tensor(out=dneg[:], in0=outt[:], in1=prev_t[:], op=OP.subtract)
                sqx = wk.tile([128, H], bf16, tag="sqx")
                nc.scalar.activation(sqx[:], dneg[:], AF.Square, bias=eps128[:], scale=-1.0,
                                     accum_out=dmat[:, s:s + 1])
            dc = wk.tile([128, NS], f32, tag="dc")
            nc.vector.tensor_scalar_min(dc[:], dmat[:], CLIP_DIST)
            ex = wk.tile([128, NS], f32, tag="ex")
            sumexp = wk.tile([128, 1], f32, tag="sumexp")
            nc.scalar.activation(ex[:], dc[:], AF.Exp, scale=-1.0, accum_out=sumexp[:])
            nc.scalar.activation(negmat[:, pt:pt + 1], sumexp[:], AF.Ln,
                                 bias=eps128[:], scale=1.0 / N)

        # ---- finalize scalars ----
        psn = pp_scan.tile([1, 8], f32, tag="scan_ps")
        nc.tensor.matmul(psn[:], lhsT=ones128f[:, :1], rhs=negmat[:], start=True, stop=True)
        scr = wk.tile([1, 8], f32, tag="scr")
        negsc = wk.tile([1, 1], f32, tag="negsc")
        nc.scalar.activation(scr[:], psn[:], AF.Identity, accum_out=negsc[:])
        nc.sync.dma_start(neg_out[:, :], negsc[:])
        psp = pp_scan.tile([1, 1], f32, tag="scan_ps")
        nc.tensor.matmul(psp[:], lhsT=ones64f[:, :1], rhs=pos_acc[:], start=True, stop=True)
        possc = wk.tile([1, 1], f32, tag="possc")
        nc.scalar.mul(possc[:], psp[:], TEMP / S)
        nc.sync.dma_start(pos_out[:, :], possc[:])

    nc.compile()
    return nc


def _get_nc():
    if "nc" not in _CACHE:
        _CACHE["nc"] = _build()
    return _CACHE["nc"]


def kernel(**inputs):
    from concourse.bass_utils import run_bass_kernel_spmd

    bf = ml_dtypes.bfloat16
    data = np.asarray(inputs["data"]).astype(np.int32)          # [S, B]
    samples = np.asarray(inputs["samples"]).astype(np.int32)    # [NS, N]
    emb_W = np.asarray(inputs["emb_W"], dtype=np.float32)
    W_ih = np.asarray(inputs["W_ih"], dtype=np.float32)
    b_ih = np.asarray(inputs["b_ih"], dtype=np.float32)
    W_hh = np.asarray(inputs["W_hh"], dtype=np.float32)
    b_hh = np.asarray(inputs["b_hh"], dtype=np.float32)

    nc = _get_nc()

    wihT = np.ascontiguousarray(W_ih.T).astype(bf)
    whhT = np.ascontiguousarray(W_hh.T).astype(bf)
    bias2 = (b_ih + b_hh).reshape(1, H).astype(bf)
    data_flat = data.reshape(N)  # t-major

    in_maps = []
    for c in range(NC):
        sl = slice(c * PSH, (c + 1) * PSH)
        samp = np.empty((NS * PSH, 1), dtype=np.int32)
        for s in range(NS):
            samp[s * PSH:(s + 1) * PSH, 0] = samples[s, sl]
        in_maps.append({
            "emb": emb_W,
            "wihT": wihT,
            "whhT": whhT,
            "bias2": bias2,
            "wx_idx": data_flat[sl].reshape(PSH, 1).astype(np.int32),
            "ps_idx": np.arange(c * VSH, (c + 1) * VSH, dtype=np.int32).reshape(VSH, 1),
            "samp_idx": samp,
            "prev_idx": np.arange(c * PSH, (c + 1) * PSH, dtype=np.int32).reshape(PSH, 1),
        })

    res = run_bass_kernel_spmd(nc, in_maps, core_ids=list(range(NC)))
    _CACHE["last_res"] = res
    pos = float(res.results[0]["pos_out"].ravel()[0])
    neg = sum(float(r["neg_out"].ravel()[0]) for r in res.results)
    return np.float32(pos + neg)


# revision 6
# speedup vs baseline: 1.1373x; 1.1373x over previous
"""Trainium2 Bass kernel for nn_RNNModel loss (RNN scan + contrastive sample loss).

Strategy (8 cores, data-parallel):
  - Project token table P' = emb @ W_ih.T + (b_ih + b_hh), sharded 4000 rows/core,
    AllGather -> full projected table (bf16). Sample "matmuls" become row gathers.
  - Wx for the scan = same projection of the 8192 data tokens, sharded 1024
    rows/core + AllGather (computed directly so the scan can start early).
  - RNN scan (128 steps, [64,1024] hidden) replicated on every core: 18 bf16
    matmuls/step accumulating Wx (identity-matmul) + U@h in PSUM, tanh on ACT,
    h transposed for the next step via DMA-transpose. Positive pairwise term
    accumulated in-scan. h trajectory stored to DRAM (bf16).
  - Negative block position-sharded: core c handles positions [1024c, 1024c+1024)
    for all 10 samples: gather prev rows, hiddens_U matmul, gather projected
    sample rows, add + tanh + squared-distance (ACT Square w/ accumulate),
    clip/exp/log reduce -> scalar partial.
  - Host sums per-core partials (pos from core 0; neg from all cores).
"""

import numpy as np
import ml_dtypes
from contextlib import ExitStack

V, H, S, B, NS, NC = 32000, 1024, 128, 64, 10, 8
N = S * B            # 8192 positions
VSH = V // NC        # 4000 table rows per core
PSH = N // NC        # 1024 positions per core
TEMP, CLIP_DIST, EPS = 65.0, 0.01, 1e-6

_CACHE = {}


def _build():
    import concourse.bass as bass
    import concourse.tile as tile
    from concourse import bacc, mybir
    from concourse.masks import make_identity

    f32 = mybir.dt.float32
    bf16 = mybir.dt.bfloat16
    i32 = mybir.dt.int32
    AF = mybir.ActivationFunctionType
    OP = mybir.AluOpType

    nc = bacc.Bacc("TRN2", target_bir_lowering=False, debug=False, num_devices=NC)

    # ---- I/O ----
    emb = nc.dram_tensor("emb", [V, H], f32, kind="ExternalInput")
    wihT = nc.dram_tensor("wihT", [H, H], bf16, kind="ExternalInput")
    whhT = nc.dram_tensor("whhT", [H, H], bf16, kind="ExternalInput")
    bias2 = nc.dram_tensor("bias2", [1, H], bf16, kind="ExternalInput")
    wx_idx = nc.dram_tensor("wx_idx", [PSH, 1], i32, kind="ExternalInput")
    ps_idx = nc.dram_tensor("ps_idx", [VSH, 1], i32, kind="ExternalInput")
    samp_idx = nc.dram_tensor("samp_idx", [NS * PSH, 1], i32, kind="ExternalInput")
    prev_idx = nc.dram_tensor("prev_idx", [PSH, 1], i32, kind="ExternalInput")
    pos_out = nc.dram_tensor("pos_out", [1, 1], f32, kind="ExternalOutput")
    neg_out = nc.dram_tensor("neg_out", [1, 1], f32, kind="ExternalOutput")

    # ---- internal DRAM ----
    wx_sh = nc.dram_tensor("wx_sh", [PSH, H], bf16)
    wx_all = nc.dram_tensor("wx_all", [N, H], bf16, addr_space="Shared")
    p_sh = nc.dram_tensor("p_sh", [VSH, H], bf16)
    p_all = nc.dram_tensor("p_all", [V, H], bf16, addr_space="Shared")
    raw = nc.dram_tensor("raw", [N, H], bf16)

    groups = [list(range(NC))]

    with tile.TileContext(nc) as tc, ExitStack() as ctx:
        const = ctx.enter_context(tc.tile_pool(name="const", bufs=1))
        io = ctx.enter_context(tc.tile_pool(name="io", bufs=4))
        wk = ctx.enter_context(tc.tile_pool(name="wk", bufs=3))
        hp = ctx.enter_context(tc.tile_pool(name="hp", bufs=2))
        pp_scan = ctx.enter_context(tc.tile_pool(name="pp_scan", bufs=2, space="PSUM"))
        pp_big = ctx.enter_context(tc.tile_pool(name="pp_big", bufs=2, space="PSUM"))

        # ---- constants / weights in SBUF ----
        wihT_sb = const.tile([128, 8 * H], bf16)
        whhT_sb = const.tile([128, 8 * H], bf16)
        for kt in range(8):
            nc.sync.dma_start(wihT_sb[:, kt * H:(kt + 1) * H], wihT[kt * 128:(kt + 1) * 128, :])
            nc.sync.dma_start(whhT_sb[:, kt * H:(kt + 1) * H], whhT[kt * 128:(kt + 1) * 128, :])
        bias2_sb = const.tile([1, H], bf16)
        nc.sync.dma_start(bias2_sb[:], bias2[:, :])
        ones1 = const.tile([1, 128], bf16)
        nc.vector.memset(ones1[:], 1.0)
        I64 = const.tile([64, 64], bf16)
        make_identity(nc, I64[:])
        ones64f = const.tile([64, 1], f32)
        nc.vector.memset(ones64f[:], 1.0)
        ones128f = const.tile([128, 1], f32)
        nc.vector.memset(ones128f[:], 1.0)
        pos_acc = const.tile([64, 1], f32)
        nc.vector.memset(pos_acc[:], 0.0)
        eps64 = const.tile([64, 1], f32)
        nc.vector.memset(eps64[:], EPS)
        eps128 = const.tile([128, 1], f32)
        nc.vector.memset(eps128[:], EPS)
        negmat = const.tile([128, 8], f32)

        # ---- projection tile: rows of emb -> rows of (e @ W_ih.T + bias2), bf16 -> dst
        def proj_tile(idx_ap, dst_ap, it, rows):
            idx_t = io.tile([128, 1], i32, tag="idx")
            nc.sync.dma_start(idx_t[:rows], idx_ap[it * 128: it * 128 + rows, :])
            ew = wk.tile([128, H], f32, tag="ew")
            nc.gpsimd.indirect_dma_start(
                out=ew[:rows], out_offset=None, in_=emb[:, :],
                in_offset=bass.IndirectOffsetOnAxis(ap=idx_t[:rows, :1], axis=0))
            ewb = wk.tile([128, H], bf16, tag="ewb")
            nc.vector.tensor_copy(ewb[:rows], ew[:rows])
            eT = wk.tile([128, 8 * 128], bf16, tag="eT")
            nc.sync.dma_start_transpose(
                out=eT[:].rearrange("p (k b) -> p k b", b=128)[:, :, :rows],
                in_=ewb[:rows, :])
            ps = pp_big.tile([128, H], f32, tag="proj_ps")
            for sl in (slice(0, 512), slice(512, 1024)):
                nc.tensor.matmul(ps[:rows, sl], lhsT=ones1[:1, :rows],
                                 rhs=bias2_sb[:1, sl], start=True, stop=False,
                                 skip_group_check=True)
            for k in range(8):
                for half in range(2):
                    sl = slice(half * 512, (half + 1) * 512)
                    nc.tensor.matmul(
                        ps[:rows, sl],
                        lhsT=eT[:, k * 128: k * 128 + rows],
                        rhs=wihT_sb[:, k * H + half * 512: k * H + (half + 1) * 512],
                        start=False, stop=(k == 7), skip_group_check=True)
            ob = wk.tile([128, H], bf16, tag="ob")
            nc.vector.tensor_copy(ob[:rows], ps[:rows])
            nc.sync.dma_start(dst_ap[it * 128: it * 128 + rows, :], ob[:rows])

        # ---- Wx shard + AllGather ----
        for it in range(PSH // 128):
            proj_tile(wx_idx, wx_sh, it, 128)
        nc.gpsimd.collective_compute(
            "AllGather", mybir.AluOpType.bypass, replica_groups=groups,
            ins=[wx_sh.ap().opt()], outs=[wx_all.ap().opt()])

        # ---- P' shard tiles (interleaved into scan below) ----
        n_ptiles = (VSH + 127) // 128  # 32 (last tile has 32 rows)

        def p_tile(i):
            rows = min(128, VSH - i * 128)
            proj_tile(ps_idx, p_sh, i, rows)

        # ---- scan init ----
        h_prev = hp.tile([64, H], bf16, tag="h")
        nc.vector.memset(h_prev[:], 0.0)
        hT_prev = hp.tile([128, 8 * 64], bf16, tag="hT")
        nc.vector.memset(hT_prev[:], 0.0)
        nc.sync.dma_start(raw[0:64, :], h_prev[:])

        # ---- scan ----
        for t in range(1, S + 1):
            # interleave projected-table tiles into the first 64 steps
            if t % 2 == 1 and (t - 1) // 2 < n_ptiles:
                p_tile((t - 1) // 2)
            wx_t = io.tile([64, H], bf16, tag="wx")
            nc.sync.dma_start(wx_t[:], wx_all[(t - 1) * 64: t * 64, :])
            ps = pp_scan.tile([64, H], f32, tag="scan_ps")
            for sl in (slice(0, 512), slice(512, 1024)):
                nc.tensor.matmul(ps[:, sl], lhsT=I64[:], rhs=wx_t[:, sl],
                                 start=True, stop=False, skip_group_check=True)
            for k in range(8):
                for half in range(2):
                    sl = slice(half * 512, (half + 1) * 512)
                    nc.tensor.matmul(
                        ps[:, sl],
                        lhsT=hT_prev[:, k * 64:(k + 1) * 64],
                        rhs=whhT_sb[:, k * H + half * 512: k * H + (half + 1) * 512],
                        start=False, stop=(k == 7), skip_group_check=True)
            h_cur = hp.tile([64, H], bf16, tag="h")
            nc.scalar.activation(h_cur[:, 0:512], ps[:, 0:512], AF.Tanh)
            nc.scalar.activation(h_cur[:, 512:1024], ps[:, 512:1024], AF.Tanh)
            # positive term: (h_{t-1} - h_t + eps)^2 summed
            d = wk.tile([64, H], bf16, tag="d")
            nc.vector.tensor_tensor(out=d[:], in0=h_prev[:], in1=h_cur[:], op=OP.subtract)
            sq = wk.tile([64, H], bf16, tag="sq")
            posc = wk.tile([64, 1], f32, tag="posc")
            nc.scalar.activation(sq[:], d[:], AF.Square, bias=eps64[:], scale=1.0,
                                 accum_out=posc[:])
            nc.vector.tensor_tensor(out=pos_acc[:], in0=pos_acc[:], in1=posc[:], op=OP.add)
            if t < S:
                nc.sync.dma_start(raw[t * 64:(t + 1) * 64, :], h_cur[:])
                hT_cur = hp.tile([128, 8 * 64], bf16, tag="hT")
                nc.sync.dma_start_transpose(
                    out=hT_cur[:, 0:256].rearrange("p (k b) -> p k b", b=64),
                    in_=h_cur[:, 0:512])
                nc.sync.dma_start_transpose(
                    out=hT_cur[:, 256:512].rearrange("p (k b) -> p k b", b=64),
                    in_=h_cur[:, 512:1024])
                hT_prev = hT_cur
            h_prev = h_cur

        # AllGather the projected table (shards all written during scan)
        nc.gpsimd.collective_compute(
            "AllGather", mybir.AluOpType.bypass, replica_groups=groups,
            ins=[p_sh.ap().opt()], outs=[p_all.ap().opt()])

        # ---- negative block: 8 position-tiles x 10 samples ----
        for pt in range(8):
            pidx_t = io.tile([128, 1], i32, tag="idx")
            nc.sync.dma_start(pidx_t[:], prev_idx[pt * 128:(pt + 1) * 128, :])
            prev_t = wk.tile([128, H], bf16, tag="prev")
            nc.gpsimd.indirect_dma_start(
                out=prev_t[:], out_offset=None, in_=raw[:, :],
                in_offset=bass.IndirectOffsetOnAxis(ap=pidx_t[:, :1], axis=0))
            prevT = wk.tile([128, 8 * 128], bf16, tag="prevT")
            nc.sync.dma_start_transpose(
                out=prevT[:].rearrange("p (k b) -> p k b", b=128),
                in_=prev_t[:])
            ps = pp_big.tile([128, H], f32, tag="proj_ps")
            for k in range(8):
                for half in range(2):
                    sl = slice(half * 512, (half + 1) * 512)
                    nc.tensor.matmul(
                        ps[:, sl],
                        lhsT=prevT[:, k * 128:(k + 1) * 128],
                        rhs=whhT_sb[:, k * H + half * 512: k * H + (half + 1) * 512],
                        start=(k == 0), stop=(k == 7), skip_group_check=True)
            hU = wk.tile([128, H], bf16, tag="hU")
            nc.vector.tensor_copy(hU[:], ps[:])
            dmat = wk.tile([128, NS], f32, tag="dmat")
            for s in range(NS):
                sidx_t = io.tile([128, 1], i32, tag="idx")
                nc.sync.dma_start(sidx_t[:], samp_idx[(s * 8 + pt) * 128:(s * 8 + pt + 1) * 128, :])
                spw = wk.tile([128, H], bf16, tag="spw")
                nc.gpsimd.indirect_dma_start(
                    out=spw[:], out_offset=None, in_=p_all[:, :],
                    in_offset=bass.IndirectOffsetOnAxis(ap=sidx_t[:, :1], axis=0))
                pre = wk.tile([128, H], bf16, tag="pre")
                nc.vector.tensor_tensor(out=pre[:], in0=spw[:], in1=hU[:], op=OP.add)
                outt = wk.tile([128, H], bf16, tag="outt")
                nc.scalar.activation(outt[:], pre[:], AF.Tanh)
                dneg = wk.tile([128, H], bf16, tag="dneg")
                nc.vector.tensor_tensor(out=dneg[:], in0=outt[:], in1=prev_t[:], op=OP.subtract)
                sqx = wk.tile([128, H], bf16, tag="sqx")
                nc.scalar.activation(sqx[:], dneg[:], AF.Square, bias=eps128[:], scale=-1.0,
                                     accum_out=dmat[:, s:s + 1])
            dc = wk.tile([128, NS], f32, tag="dc")
            nc.vector.tensor_scalar_min(dc[:], dmat[:], CLIP_DIST)
            ex = wk.tile([128, NS], f32, tag="ex")
            sumexp = wk.tile([128, 1], f32, tag="sumexp")
            nc.scalar.activation(ex[:], dc[:], AF.Exp, scale=-1.0, accum_out=sumexp[:])
            nc.scalar.activation(negmat[:, pt:pt + 1], sumexp[:], AF.Ln,
                                 bias=eps128[:], scale=1.0 / N)

        # ---- finalize scalars ----
        psn = pp_scan.tile([1, 8], f32, tag="scan_ps")
        nc.tensor.matmul(psn[:], lhsT=ones128f[:, :1], rhs=negmat[:], start=True, stop=True)
        scr = wk.tile([1, 8], f32, tag="scr")
        negsc = wk.tile([1, 1], f32, tag="negsc")
        nc.scalar.activation(scr[:], psn[:], AF.Identity, accum_out=negsc[:])
        nc.sync.dma_start(neg_out[:, :], negsc[:])
        psp = pp_scan.tile([1, 1], f32, tag="scan_ps")
        nc.tensor.matmul(psp[:], lhsT=ones64f[:, :1], rhs=pos_acc[:], start=True, stop=True)
        possc = wk.tile([1, 1], f32, tag="possc")
        nc.scalar.mul(possc[:], psp[:], TEMP / S)
        nc.sync.dma_start(pos_out[:, :], possc[:])

    nc.compile()
    return nc


def _get_nc():
    if "nc" not in _CACHE:
        _CACHE["nc"] = _build()
    return _CACHE["nc"]


def kernel(**inputs):
    from concourse.bass_utils import run_bass_kernel_spmd

    bf = ml_dtypes.bfloat16
    data = np.asarray(inputs["data"]).astype(np.int32)          # [S, B]
    samples = np.asarray(inputs["samples"]).astype(np.int32)    # [NS, N]
    emb_W = np.asarray(inputs["emb_W"], dtype=np.float32)
    W_ih = np.asarray(inputs["W_ih"], dtype=np.float32)
    b_ih = np.asarray(inputs["b_ih"], dtype=np.float32)
    W_hh = np.asarray(inputs["W_hh"], dtype=np.float32)
    b_hh = np.asarray(inputs["b_hh"], dtype=np.float32)

    nc = _get_nc()

    wihT = np.ascontiguousarray(W_ih.T).astype(bf)
    whhT = np.ascontiguousarray(W_hh.T).astype(bf)
    bias2 = (b_ih + b_hh).reshape(1, H).astype(bf)
    data_flat = data.reshape(N)  # t-major

    in_maps = []
    for c in range(NC):
        sl = slice(c * PSH, (c + 1) * PSH)
        samp = np.empty((NS * PSH, 1), dtype=np.int32)
        for s in range(NS):
            samp[s * PSH:(s + 1) * PSH, 0] = samples[s, sl]
        in_maps.append({
            "emb": emb_W,
            "wihT": wihT,
            "whhT": whhT,
            "bias2": bias2,
            "wx_idx": data_flat[sl].reshape(PSH, 1).astype(np.int32),
            "ps_idx": np.arange(c * VSH, (c + 1) * VSH, dtype=np.int32).reshape(VSH, 1),
            "samp_idx": samp,
            "prev_idx": np.arange(c * PSH, (c + 1) * PSH, dtype=np.int32).reshape(PSH, 1),
        })

    res = run_bass_kernel_spmd(nc, in_maps, core_ids=list(range(NC)))
    _CACHE["last_res"] = res
    pos = float(res.results[0]["pos_out"].ravel()[0])
    neg = sum(float(r["neg_out"].ravel()[0]) for r in res.results)
    return np.float32(pos + neg)


# revision 9
# speedup vs baseline: 1.2056x; 1.0601x over previous
"""Trainium2 Bass kernel for nn_RNNModel loss (RNN scan + contrastive sample loss).

Strategy (8 cores, data-parallel):
  - Project token table P' = emb @ W_ih.T + (b_ih + b_hh), sharded 4000 rows/core,
    AllGather -> full projected table (bf16). Sample "matmuls" become row gathers.
  - Wx for the scan = same projection of the 8192 data tokens, sharded 1024
    rows/core + AllGather (computed directly so the scan can start early).
  - RNN scan (128 steps, [64,1024] hidden) replicated on every core: 18 bf16
    matmuls/step accumulating Wx (identity-matmul) + U@h in PSUM, tanh on ACT,
    h transposed for the next step via DMA-transpose. Positive pairwise term
    accumulated in-scan. h trajectory stored to DRAM (bf16).
  - Negative block position-sharded: core c handles positions [1024c, 1024c+1024)
    for all 10 samples: gather prev rows, hiddens_U matmul, gather projected
    sample rows, add + tanh + squared-distance (ACT Square w/ accumulate),
    clip/exp/log reduce -> scalar partial.
  - Host sums per-core partials (pos from core 0; neg from all cores).
"""

import numpy as np
import ml_dtypes
from contextlib import ExitStack

V, H, S, B, NS, NC = 32000, 1024, 128, 64, 10, 8
N = S * B            # 8192 positions
VSH = V // NC        # 4000 table rows per core
PSH = N // NC        # 1024 positions per core
TEMP, CLIP_DIST, EPS = 65.0, 0.01, 1e-6

_CACHE = {}


def _build():
    import concourse.bass as bass
    import concourse.tile as tile
    from concourse import bacc, mybir
    from concourse.masks import make_identity

    f32 = mybir.dt.float32
    bf16 = mybir.dt.bfloat16
    i32 = mybir.dt.int32
    AF = mybir.ActivationFunctionType
    OP = mybir.AluOpType

    nc = bacc.Bacc("TRN2", target_bir_lowering=False, debug=False, num_devices=NC)

    # ---- I/O ----
    emb = nc.dram_tensor("emb", [V, H], f32, kind="ExternalInput")
    wihT = nc.dram_tensor("wihT", [H, H], bf16, kind="ExternalInput")
    whhT = nc.dram_tensor("whhT", [H, H], bf16, kind="ExternalInput")
    bias2 = nc.dram_tensor("bias2", [1, H], bf16, kind="ExternalInput")
    wx_idx = nc.dram_tensor("wx_idx", [PSH, 1], i32, kind="ExternalInput")
    ps_idx = nc.dram_tensor("ps_idx", [VSH, 1], i32, kind="ExternalInput")
    samp_idx = nc.dram_tensor("samp_idx", [128, 80], i32, kind="ExternalInput")
    prev_idx = nc.dram_tensor("prev_idx", [128, 8], i32, kind="ExternalInput")
    pos_out = nc.dram_tensor("pos_out", [1, 1], f32, kind="ExternalOutput")
    neg_out = nc.dram_tensor("neg_out", [1, 1], f32, kind="ExternalOutput")

    # ---- internal DRAM ----
    wx_sh = nc.dram_tensor("wx_sh", [PSH, H], bf16)
    wx_all = nc.dram_tensor("wx_all", [N, H], bf16, addr_space="Shared")
    p_sh = nc.dram_tensor("p_sh", [VSH, H], bf16)
    p_all = nc.dram_tensor("p_all", [V, H], bf16, addr_space="Shared")
    raw = nc.dram_tensor("raw", [N, H], bf16)

    groups = [list(range(NC))]

    with tile.TileContext(nc) as tc, ExitStack() as ctx:
        const = ctx.enter_context(tc.tile_pool(name="const", bufs=1))
        io = ctx.enter_context(tc.tile_pool(name="io", bufs=4))
        wk = ctx.enter_context(tc.tile_pool(name="wk", bufs=3))
        hp = ctx.enter_context(tc.tile_pool(name="hp", bufs=3))
        pp_scan = ctx.enter_context(tc.tile_pool(name="pp_scan", bufs=2, space="PSUM"))
        pp_big = ctx.enter_context(tc.tile_pool(name="pp_big", bufs=2, space="PSUM"))

        # ---- constants / weights in SBUF ----
        wihT_sb = const.tile([128, 8 * H], bf16)
        whhT_sb = const.tile([128, 8 * H], bf16)
        for kt in range(8):
            nc.sync.dma_start(wihT_sb[:, kt * H:(kt + 1) * H], wihT[kt * 128:(kt + 1) * 128, :])
            nc.sync.dma_start(whhT_sb[:, kt * H:(kt + 1) * H], whhT[kt * 128:(kt + 1) * 128, :])
        bias2_sb = const.tile([1, H], bf16)
        nc.sync.dma_start(bias2_sb[:], bias2[:, :])
        ones1 = const.tile([1, 128], bf16)
        nc.vector.memset(ones1[:], 1.0)
        I64 = const.tile([64, 64], bf16)
        make_identity(nc, I64[:])
        ones64f = const.tile([64, 1], f32)
        nc.vector.memset(ones64f[:], 1.0)
        ones128f = const.tile([128, 1], f32)
        nc.vector.memset(ones128f[:], 1.0)
        pos_acc = const.tile([64, 1], f32)
        nc.vector.memset(pos_acc[:], 0.0)
        eps64 = const.tile([64, 1], f32)
        nc.vector.memset(eps64[:], EPS)
        eps128 = const.tile([128, 1], f32)
        nc.vector.memset(eps128[:], EPS)
        negmat = const.tile([128, 8], f32)

        # ---- projection tile: rows of emb -> rows of (e @ W_ih.T + bias2), bf16 -> dst
        def proj_tile(idx_ap, dst_ap, it, rows):
            idx_t = io.tile([128, 1], i32, tag="idx")
            nc.sync.dma_start(idx_t[:rows], idx_ap[it * 128: it * 128 + rows, :])
            ew = wk.tile([128, H], f32, tag="ew")
            nc.gpsimd.indirect_dma_start(
                out=ew[:rows], out_offset=None, in_=emb[:, :],
                in_offset=bass.IndirectOffsetOnAxis(ap=idx_t[:rows, :1], axis=0))
            ewb = wk.tile([128, H], bf16, tag="ewb")
            nc.vector.tensor_copy(ewb[:rows], ew[:rows])
            eT = wk.tile([128, 8 * 128], bf16, tag="eT")
            nc.sync.dma_start_transpose(
                out=eT[:].rearrange("p (k b) -> p k b", b=128)[:, :, :rows],
                in_=ewb[:rows, :])
            ps = pp_big.tile([128, H], f32, tag="proj_ps")
            for sl in (slice(0, 512), slice(512, 1024)):
                nc.tensor.matmul(ps[:rows, sl], lhsT=ones1[:1, :rows],
                                 rhs=bias2_sb[:1, sl], start=True, stop=False,
                                 skip_group_check=True)
            for k in range(8):
                for half in range(2):
                    sl = slice(half * 512, (half + 1) * 512)
                    nc.tensor.matmul(
                        ps[:rows, sl],
                        lhsT=eT[:, k * 128: k * 128 + rows],
                        rhs=wihT_sb[:, k * H + half * 512: k * H + (half + 1) * 512],
                        start=False, stop=(k == 7), skip_group_check=True)
            ob = wk.tile([128, H], bf16, tag="ob")
            nc.vector.tensor_copy(ob[:rows], ps[:rows])
            nc.sync.dma_start(dst_ap[it * 128: it * 128 + rows, :], ob[:rows])

        # ---- Wx shard + AllGather ----
        for it in range(PSH // 128):
            proj_tile(wx_idx, wx_sh, it, 128)
        nc.gpsimd.collective_compute(
            "AllGather", mybir.AluOpType.bypass, replica_groups=groups,
            ins=[wx_sh.ap().opt()], outs=[wx_all.ap().opt()])

        # ---- P' shard tiles (interleaved into scan below) ----
        n_ptiles = (VSH + 127) // 128  # 32 (last tile has 32 rows)

        def p_tile(i):
            rows = min(128, VSH - i * 128)
            proj_tile(ps_idx, p_sh, i, rows)

        # ---- scan init ----
        h_prev = hp.tile([64, H], bf16, tag="h")
        nc.vector.memset(h_prev[:], 0.0)
        hT_prev = hp.tile([128, 8 * 64], bf16, tag="hT")
        nc.vector.memset(hT_prev[:], 0.0)
        nc.sync.dma_start(raw[0:64, :], h_prev[:])

        # ---- scan ----
        for t in range(1, S + 1):
            # interleave projected-table tiles into the first 64 steps
            if t % 2 == 1 and (t - 1) // 2 < n_ptiles:
                p_tile((t - 1) // 2)
            wx_t = io.tile([64, H], bf16, tag="wx")
            nc.sync.dma_start(wx_t[:], wx_all[(t - 1) * 64: t * 64, :])
            h_cur = hp.tile([64, H], bf16, tag="h")
            if t < S:
                hT_cur = hp.tile([128, 8 * 64], bf16, tag="hT")
            else:
                hT_cur = None
            ps = pp_scan.tile([64, H], f32, tag="scan_ps")
            for half in range(2):
                sl = slice(half * 512, (half + 1) * 512)
                nc.tensor.matmul(ps[:, sl], lhsT=I64[:], rhs=wx_t[:, sl],
                                 start=True, stop=False, skip_group_check=True)
                for k in range(8):
                    nc.tensor.matmul(
                        ps[:, sl],
                        lhsT=hT_prev[:, k * 64:(k + 1) * 64],
                        rhs=whhT_sb[:, k * H + half * 512: k * H + (half + 1) * 512],
                        start=False, stop=(k == 7), skip_group_check=True)
                nc.scalar.activation(h_cur[:, sl], ps[:, sl], AF.Tanh)
                if t < S:
                    nc.sync.dma_start_transpose(
                        out=hT_cur[:, half * 256:(half + 1) * 256].rearrange(
                            "p (k b) -> p k b", b=64),
                        in_=h_cur[:, sl])

            # positive term: (h_{t-1} - h_t + eps)^2 summed
            d = wk.tile([64, H], bf16, tag="d")
            nc.vector.tensor_tensor(out=d[:], in0=h_prev[:], in1=h_cur[:], op=OP.subtract)
            sq = wk.tile([64, H], bf16, tag="sq")
            posc = wk.tile([64, 1], f32, tag="posc")
            nc.scalar.activation(sq[:], d[:], AF.Square, bias=eps64[:], scale=1.0,
                                 accum_out=posc[:])
            nc.vector.tensor_tensor(out=pos_acc[:], in0=pos_acc[:], in1=posc[:], op=OP.add)
            if t < S:
                nc.sync.dma_start(raw[t * 64:(t + 1) * 64, :], h_cur[:])
                hT_prev = hT_cur
            h_prev = h_cur

        # AllGather the projected table (shards all written during scan)
        nc.gpsimd.collective_compute(
            "AllGather", mybir.AluOpType.bypass, replica_groups=groups,
            ins=[p_sh.ap().opt()], outs=[p_all.ap().opt()])

        # ---- negative block: 8 position-tiles x 10 samples ----
        sidx_all = const.tile([128, 80], i32)
        nc.sync.dma_start(sidx_all[:], samp_idx[:, :])
        pidx_all = const.tile([128, 8], i32)
        nc.sync.dma_start(pidx_all[:], prev_idx[:, :])
        for pt in range(8):
            prev_t = wk.tile([128, H], bf16, tag="prev")
            nc.gpsimd.indirect_dma_start(
                out=prev_t[:], out_offset=None, in_=raw[:, :],
                in_offset=bass.IndirectOffsetOnAxis(ap=pidx_all[:, pt:pt + 1], axis=0))
            prevT = wk.tile([128, 8 * 128], bf16, tag="prevT")
            nc.sync.dma_start_transpose(
                out=prevT[:].rearrange("p (k b) -> p k b", b=128),
                in_=prev_t[:])
            ps = pp_big.tile([128, H], f32, tag="proj_ps")
            for k in range(8):
                for half in range(2):
                    sl = slice(half * 512, (half + 1) * 512)
                    nc.tensor.matmul(
                        ps[:, sl],
                        lhsT=prevT[:, k * 128:(k + 1) * 128],
                        rhs=whhT_sb[:, k * H + half * 512: k * H + (half + 1) * 512],
                        start=(k == 0), stop=(k == 7), skip_group_check=True)
            hU = wk.tile([128, H], bf16, tag="hU")
            nc.vector.tensor_copy(hU[:], ps[:])
            dmat = wk.tile([128, NS], f32, tag="dmat")
            for s in range(NS):
                spw = wk.tile([128, H], bf16, tag="spw")
                nc.gpsimd.indirect_dma_start(
                    out=spw[:], out_offset=None, in_=p_all[:, :],
                    in_offset=bass.IndirectOffsetOnAxis(ap=sidx_all[:, s * 8 + pt: s * 8 + pt + 1], axis=0))
                pre = wk.tile([128, H], bf16, tag="pre")
                nc.vector.tensor_tensor(out=pre[:], in0=spw[:], in1=hU[:], op=OP.add)
                outt = wk.tile([128, H], bf16, tag="outt")
                nc.scalar.activation(outt[:], pre[:], AF.Tanh)
                dneg = wk.tile([128, H], bf16, tag="dneg")
                nc.vector.tensor_tensor(out=dneg[:], in0=outt[:], in1=prev_t[:], op=OP.subtract)
                sqx = wk.tile([128, H], bf16, tag="sqx")
                nc.scalar.activation(sqx[:], dneg[:], AF.Square, bias=eps128[:], scale=-1.0,
                                     accum_out=dmat[:, s:s + 1])
            dc = wk.tile([128, NS], f32, tag="dc")
            nc.vector.tensor_scalar_min(dc[:], dmat[:], CLIP_DIST)
            ex = wk.tile([128, NS], f32, tag="ex")
            sumexp = wk.tile([128, 1], f32, tag="sumexp")
            nc.scalar.activation(ex[:], dc[:], AF.Exp, scale=-1.0, accum_out=sumexp[:])
            nc.scalar.activation(negmat[:, pt:pt + 1], sumexp[:], AF.Ln,
                                 bias=eps128[:], scale=1.0 / N)

        # ---- finalize scalars ----
        psn = pp_scan.tile([1, 8], f32, tag="scan_ps")
        nc.tensor.matmul(psn[:], lhsT=ones128f[:, :1], rhs=negmat[:], start=True, stop=True)
        scr = wk.tile([1, 8], f32, tag="scr")
        negsc = wk.tile([1, 1], f32, tag="negsc")
        nc.scalar.activation(scr[:], psn[:], AF.Identity, accum_out=negsc[:])
        nc.sync.dma_start(neg_out[:, :], negsc[:])
        psp = pp_scan.tile([1, 1], f32, tag="scan_ps")
        nc.tensor.matmul(psp[:], lhsT=ones64f[:, :1], rhs=pos_acc[:], start=True, stop=True)
        possc = wk.tile([1, 1], f32, tag="possc")
        nc.scalar.mul(possc[:], psp[:], TEMP / S)
        nc.sync.dma_start(pos_out[:, :], possc[:])

    nc.compile()
    return nc


def _get_nc():
    if "nc" not in _CACHE:
        _CACHE["nc"] = _build()
    return _CACHE["nc"]


def kernel(**inputs):
    from concourse.bass_utils import run_bass_kernel_spmd

    bf = ml_dtypes.bfloat16
    data = np.asarray(inputs["data"]).astype(np.int32)          # [S, B]
    samples = np.asarray(inputs["samples"]).astype(np.int32)    # [NS, N]
    emb_W = np.asarray(inputs["emb_W"], dtype=np.float32)
    W_ih = np.asarray(inputs["W_ih"], dtype=np.float32)
    b_ih = np.asarray(inputs["b_ih"], dtype=np.float32)
    W_hh = np.asarray(inputs["W_hh"], dtype=np.float32)
    b_hh = np.asarray(inputs["b_hh"], dtype=np.float32)

    nc = _get_nc()

    wihT = np.ascontiguousarray(W_ih.T).astype(bf)
    whhT = np.ascontiguousarray(W_hh.T).astype(bf)
    bias2 = (b_ih + b_hh).reshape(1, H).astype(bf)
    data_flat = data.reshape(N)  # t-major

    in_maps = []
    for c in range(NC):
        sl = slice(c * PSH, (c + 1) * PSH)
        samp = np.empty((128, 80), dtype=np.int32)
        for s in range(NS):
            for pt in range(8):
                samp[:, s * 8 + pt] = samples[s, c * PSH + pt * 128: c * PSH + (pt + 1) * 128]
        in_maps.append({
            "emb": emb_W,
            "wihT": wihT,
            "whhT": whhT,
            "bias2": bias2,
            "wx_idx": data_flat[sl].reshape(PSH, 1).astype(np.int32),
            "ps_idx": np.arange(c * VSH, (c + 1) * VSH, dtype=np.int32).reshape(VSH, 1),
            "samp_idx": samp,
            "prev_idx": np.arange(c * PSH, (c + 1) * PSH, dtype=np.int32).reshape(8, 128).T.copy(),
        })

    res = run_bass_kernel_spmd(nc, in_maps, core_ids=list(range(NC)))
    _CACHE["last_res"] = res
    pos = float(res.results[0]["pos_out"].ravel()[0])
    neg = sum(float(r["neg_out"].ravel()[0]) for r in res.results)
    return np.float32(pos + neg)


# revision 11
# speedup vs baseline: 1.4716x; 1.2206x over previous
"""Trainium2 Bass kernel for nn_RNNModel loss (RNN scan + contrastive sample loss).

Strategy (8 cores, data-parallel):
  - Project token table P' = emb @ W_ih.T + (b_ih + b_hh), sharded 4000 rows/core,
    AllGather -> full projected table (bf16). Sample "matmuls" become row gathers.
  - Wx for the scan = same projection of the 8192 data tokens, sharded 1024
    rows/core + AllGather (computed directly so the scan can start early).
  - RNN scan (128 steps, [64,1024] hidden) replicated on every core: 18 bf16
    matmuls/step accumulating Wx (identity-matmul) + U@h in PSUM, tanh on ACT,
    h transposed for the next step via DMA-transpose. Positive pairwise term
    accumulated in-scan. h trajectory stored to DRAM (bf16).
  - Negative block position-sharded: core c handles positions [1024c, 1024c+1024)
    for all 10 samples: gather prev rows, hiddens_U matmul, gather projected
    sample rows, add + tanh + squared-distance (ACT Square w/ accumulate),
    clip/exp/log reduce -> scalar partial.
  - Host sums per-core partials (pos from core 0; neg from all cores).
"""

import numpy as np
import ml_dtypes
from contextlib import ExitStack

V, H, S, B, NS, NC = 32000, 1024, 128, 64, 10, 8
N = S * B            # 8192 positions
VSH = V // NC        # 4000 table rows per core
PSH = N // NC        # 1024 positions per core
TEMP, CLIP_DIST, EPS = 65.0, 0.01, 1e-6

_CACHE = {}


def _build():
    import concourse.bass as bass
    import concourse.tile as tile
    from concourse import bacc, mybir
    from concourse.masks import make_identity

    f32 = mybir.dt.float32
    bf16 = mybir.dt.bfloat16
    i32 = mybir.dt.int32
    AF = mybir.ActivationFunctionType
    OP = mybir.AluOpType

    nc = bacc.Bacc("TRN2", target_bir_lowering=False, debug=False, num_devices=NC)

    # ---- I/O ----
    emb = nc.dram_tensor("emb", [V, H], f32, kind="ExternalInput")
    wihT = nc.dram_tensor("wihT", [H, H], bf16, kind="ExternalInput")
    whhT = nc.dram_tensor("whhT", [H, H], bf16, kind="ExternalInput")
    bias2 = nc.dram_tensor("bias2", [1, H], bf16, kind="ExternalInput")
    wx_idx = nc.dram_tensor("wx_idx", [PSH, 1], i32, kind="ExternalInput")
    ps_idx = nc.dram_tensor("ps_idx", [VSH, 1], i32, kind="ExternalInput")
    samp_idx = nc.dram_tensor("samp_idx", [128, 80], i32, kind="ExternalInput")
    prev_idx = nc.dram_tensor("prev_idx", [128, 8], i32, kind="ExternalInput")
    pos_out = nc.dram_tensor("pos_out", [1, 1], f32, kind="ExternalOutput")
    neg_out = nc.dram_tensor("neg_out", [1, 1], f32, kind="ExternalOutput")

    # ---- internal DRAM ----
    wx_sh = nc.dram_tensor("wx_sh", [PSH, H], bf16)
    wx_all = nc.dram_tensor("wx_all", [N, H], bf16, addr_space="Shared")
    p_sh = nc.dram_tensor("p_sh", [VSH, H], bf16)
    p_all = nc.dram_tensor("p_all", [V, H], bf16, addr_space="Shared")
    raw = nc.dram_tensor("raw", [N, H], bf16)

    groups = [list(range(NC))]

    with tile.TileContext(nc) as tc, ExitStack() as ctx:
        const = ctx.enter_context(tc.tile_pool(name="const", bufs=1))
        io = ctx.enter_context(tc.tile_pool(name="io", bufs=4))
        wk = ctx.enter_context(tc.tile_pool(name="wk", bufs=3))
        hp = ctx.enter_context(tc.tile_pool(name="hp", bufs=3))
        pp_scan = ctx.enter_context(tc.tile_pool(name="pp_scan", bufs=2, space="PSUM"))
        pp_big = ctx.enter_context(tc.tile_pool(name="pp_big", bufs=1, space="PSUM"))

        # ---- constants / weights in SBUF ----
        wihT_sb = const.tile([128, 8 * H], bf16)
        whhT_sb = const.tile([128, 8 * H], bf16)
        for kt in range(8):
            nc.sync.dma_start(wihT_sb[:, kt * H:(kt + 1) * H], wihT[kt * 128:(kt + 1) * 128, :])
            nc.sync.dma_start(whhT_sb[:, kt * H:(kt + 1) * H], whhT[kt * 128:(kt + 1) * 128, :])
        bias2_sb = const.tile([1, H], bf16)
        nc.sync.dma_start(bias2_sb[:], bias2[:, :])
        ones1 = const.tile([1, 128], bf16)
        nc.vector.memset(ones1[:], 1.0)
        I64 = const.tile([64, 64], bf16)
        make_identity(nc, I64[:])
        ones64f = const.tile([64, 1], f32)
        nc.vector.memset(ones64f[:], 1.0)
        ones128f = const.tile([128, 1], f32)
        nc.vector.memset(ones128f[:], 1.0)
        pos_acc = const.tile([64, 1], f32)
        nc.vector.memset(pos_acc[:], 0.0)
        eps64 = const.tile([64, 1], f32)
        nc.vector.memset(eps64[:], EPS)
        eps128 = const.tile([128, 1], f32)
        nc.vector.memset(eps128[:], EPS)
        negmat = const.tile([128, 8], f32)

        # ---- projection tile: rows of emb -> rows of (e @ W_ih.T + bias2), bf16 -> dst
        def proj_tile(idx_ap, dst_ap, it, rows):
            idx_t = io.tile([128, 1], i32, tag="idx")
            nc.sync.dma_start(idx_t[:rows], idx_ap[it * 128: it * 128 + rows, :])
            ew = wk.tile([128, H], f32, tag="ew")
            nc.gpsimd.indirect_dma_start(
                out=ew[:rows], out_offset=None, in_=emb[:, :],
                in_offset=bass.IndirectOffsetOnAxis(ap=idx_t[:rows, :1], axis=0))
            ewb = wk.tile([128, H], bf16, tag="ewb")
            nc.vector.tensor_copy(ewb[:rows], ew[:rows])
            eT = wk.tile([128, 8 * 128], bf16, tag="eT")
            nc.sync.dma_start_transpose(
                out=eT[:].rearrange("p (k b) -> p k b", b=128)[:, :, :rows],
                in_=ewb[:rows, :])
            ps = pp_big.tile([128, H], f32, tag="proj_ps")
            for sl in (slice(0, 512), slice(512, 1024)):
                nc.tensor.matmul(ps[:rows, sl], lhsT=ones1[:1, :rows],
                                 rhs=bias2_sb[:1, sl], start=True, stop=False,
                                 skip_group_check=True)
            for k in range(8):
                for half in range(2):
                    sl = slice(half * 512, (half + 1) * 512)
                    nc.tensor.matmul(
                        ps[:rows, sl],
                        lhsT=eT[:, k * 128: k * 128 + rows],
                        rhs=wihT_sb[:, k * H + half * 512: k * H + (half + 1) * 512],
                        start=False, stop=(k == 7), skip_group_check=True)
            ob = wk.tile([128, H], bf16, tag="ob")
            nc.vector.tensor_copy(ob[:rows], ps[:rows])
            nc.sync.dma_start(dst_ap[it * 128: it * 128 + rows, :], ob[:rows])

        # ---- Wx shard + AllGather ----
        for it in range(PSH // 128):
            proj_tile(wx_idx, wx_sh, it, 128)
        nc.gpsimd.collective_compute(
            "AllGather", mybir.AluOpType.bypass, replica_groups=groups,
            ins=[wx_sh.ap().opt()], outs=[wx_all.ap().opt()])

        # ---- P' shard tiles (interleaved into scan below) ----
        n_ptiles = (VSH + 127) // 128  # 32 (last tile has 32 rows)

        def p_tile(i):
            rows = min(128, VSH - i * 128)
            proj_tile(ps_idx, p_sh, i, rows)

        # ---- scan init ----
        h_prev = hp.tile([64, H], bf16, tag="h")
        nc.vector.memset(h_prev[:], 0.0)
        hT_prev = hp.tile([128, 8 * 64], bf16, tag="hT")
        nc.vector.memset(hT_prev[:], 0.0)
        nc.sync.dma_start(raw[0:64, :], h_prev[:])

        # ---- scan ----
        for t in range(1, S + 1):
            # interleave projected-table tiles into the first 64 steps
            if t % 2 == 1 and (t - 1) // 2 < n_ptiles:
                p_tile((t - 1) // 2)
            wx_t = io.tile([64, H], bf16, tag="wx")
            nc.sync.dma_start(wx_t[:], wx_all[(t - 1) * 64: t * 64, :])
            h_cur = hp.tile([64, H], bf16, tag="h")
            if t < S:
                hT_cur = hp.tile([128, 8 * 64], bf16, tag="hT")
            else:
                hT_cur = None
            ps = pp_scan.tile([64, H], f32, tag="scan_ps")
            for half in range(2):
                sl = slice(half * 512, (half + 1) * 512)
                nc.tensor.matmul(ps[:, sl], lhsT=I64[:], rhs=wx_t[:, sl],
                                 start=True, stop=False, skip_group_check=True)
                for k in range(8):
                    nc.tensor.matmul(
                        ps[:, sl],
                        lhsT=hT_prev[:, k * 64:(k + 1) * 64],
                        rhs=whhT_sb[:, k * H + half * 512: k * H + (half + 1) * 512],
                        start=False, stop=(k == 7), skip_group_check=True)
                nc.scalar.activation(h_cur[:, sl], ps[:, sl], AF.Tanh)
                if t < S:
                    if half == 0:
                        nc.sync.dma_start_transpose(
                            out=hT_cur[:, 0:256].rearrange("p (k b) -> p k b", b=64),
                            in_=h_cur[:, sl])
                    else:
                        trp = pp_scan.tile([128, 256], bf16, tag="trp")
                        for k in range(4, 8):
                            nc.tensor.transpose(
                                trp[:, (k - 4) * 64:(k - 3) * 64],
                                in_=h_cur[:, k * 128:(k + 1) * 128],
                                identity=I64[:])
                        nc.vector.tensor_copy(hT_cur[:, 256:512], trp[:])

            # positive term: (h_{t-1} - h_t + eps)^2 summed
            d = wk.tile([64, H], bf16, tag="d")
            nc.vector.tensor_tensor(out=d[:], in0=h_prev[:], in1=h_cur[:], op=OP.subtract)
            sq = wk.tile([64, H], bf16, tag="sq")
            posc = wk.tile([64, 1], f32, tag="posc")
            nc.scalar.activation(sq[:], d[:], AF.Square, bias=eps64[:], scale=1.0,
                                 accum_out=posc[:])
            nc.vector.tensor_tensor(out=pos_acc[:], in0=pos_acc[:], in1=posc[:], op=OP.add)
            if t < S:
                nc.sync.dma_start(raw[t * 64:(t + 1) * 64, :], h_cur[:])
                hT_prev = hT_cur
            h_prev = h_cur

        # AllGather the projected table (shards all written during scan)
        nc.gpsimd.collective_compute(
            "AllGather", mybir.AluOpType.bypass, replica_groups=groups,
            ins=[p_sh.ap().opt()], outs=[p_all.ap().opt()])

        # ---- negative block: 8 position-tiles x 10 samples ----
        sidx_all = const.tile([128, 80], i32)
        nc.sync.dma_start(sidx_all[:], samp_idx[:, :])
        pidx_all = const.tile([128, 8], i32)
        nc.sync.dma_start(pidx_all[:], prev_idx[:, :])
        for pt in range(8):
            prev_t = wk.tile([128, H], bf16, tag="prev")
            nc.gpsimd.indirect_dma_start(
                out=prev_t[:], out_offset=None, in_=raw[:, :],
                in_offset=bass.IndirectOffsetOnAxis(ap=pidx_all[:, pt:pt + 1], axis=0))
            prevT = wk.tile([128, 8 * 128], bf16, tag="prevT")
            nc.sync.dma_start_transpose(
                out=prevT[:].rearrange("p (k b) -> p k b", b=128),
                in_=prev_t[:])
            ps = pp_big.tile([128, H], f32, tag="proj_ps")
            for k in range(8):
                for half in range(2):
                    sl = slice(half * 512, (half + 1) * 512)
                    nc.tensor.matmul(
                        ps[:, sl],
                        lhsT=prevT[:, k * 128:(k + 1) * 128],
                        rhs=whhT_sb[:, k * H + half * 512: k * H + (half + 1) * 512],
                        start=(k == 0), stop=(k == 7), skip_group_check=True)
            hU = wk.tile([128, H], bf16, tag="hU")
            nc.vector.tensor_copy(hU[:], ps[:])
            dmat = wk.tile([128, NS], f32, tag="dmat")
            for s in range(NS):
                spw = wk.tile([128, H], bf16, tag="spw")
                nc.gpsimd.indirect_dma_start(
                    out=spw[:], out_offset=None, in_=p_all[:, :],
                    in_offset=bass.IndirectOffsetOnAxis(ap=sidx_all[:, s * 8 + pt: s * 8 + pt + 1], axis=0))
                pre = wk.tile([128, H], bf16, tag="pre")
                nc.vector.tensor_tensor(out=pre[:], in0=spw[:], in1=hU[:], op=OP.add)
                outt = wk.tile([128, H], bf16, tag="outt")
                nc.scalar.activation(outt[:], pre[:], AF.Tanh)
                dneg = wk.tile([128, H], bf16, tag="dneg")
                nc.vector.tensor_tensor(out=dneg[:], in0=outt[:], in1=prev_t[:], op=OP.subtract)
                sqx = wk.tile([128, H], bf16, tag="sqx")
                nc.scalar.activation(sqx[:], dneg[:], AF.Square, bias=eps128[:], scale=-1.0,
                                     accum_out=dmat[:, s:s + 1])
            dc = wk.tile([128, NS], f32, tag="dc")
            nc.vector.tensor_scalar_min(dc[:], dmat[:], CLIP_DIST)
            ex = wk.tile([128, NS], f32, tag="ex")
            sumexp = wk.tile([128, 1], f32, tag="sumexp")
            nc.scalar.activation(ex[:], dc[:], AF.Exp, scale=-1.0, accum_out=sumexp[:])
            nc.scalar.activation(negmat[:, pt:pt + 1], sumexp[:], AF.Ln,
                                 bias=eps128[:], scale=1.0 / N)

        # ---- finalize scalars ----
        psn = pp_scan.tile([1, 8], f32, tag="scan_ps")
        nc.tensor.matmul(psn[:], lhsT=ones128f[:, :1], rhs=negmat[:], start=True, stop=True)
        scr = wk.tile([1, 8], f32, tag="scr")
        negsc = wk.tile([1, 1], f32, tag="negsc")
        nc.scalar.activation(scr[:], psn[:], AF.Identity, accum_out=negsc[:])
        nc.sync.dma_start(neg_out[:, :], negsc[:])
        psp = pp_scan.tile([1, 1], f32, tag="scan_ps")
        nc.tensor.matmul(psp[:], lhsT=ones64f[:, :1], rhs=pos_acc[:], start=True, stop=True)
        possc = wk.tile([1, 1], f32, tag="possc")
        nc.scalar.mul(possc[:], psp[:], TEMP / S)
        nc.sync.dma_start(pos_out[:, :], possc[:])

    nc.compile()
    return nc


def _get_nc():
    if "nc" not in _CACHE:
        _CACHE["nc"] = _build()
    return _CACHE["nc"]


def kernel(**inputs):
    from concourse.bass_utils import run_bass_kernel_spmd

    bf = ml_dtypes.bfloat16
    data = np.asarray(inputs["data"]).astype(np.int32)          # [S, B]
    samples = np.asarray(inputs["samples"]).astype(np.int32)    # [NS, N]
    emb_W = np.asarray(inputs["emb_W"], dtype=np.float32)
    W_ih = np.asarray(inputs["W_ih"], dtype=np.float32)
    b_ih = np.asarray(inputs["b_ih"], dtype=np.float32)
    W_hh = np.asarray(inputs["W_hh"], dtype=np.float32)
    b_hh = np.asarray(inputs["b_hh"], dtype=np.float32)

    nc = _get_nc()

    wihT = np.ascontiguousarray(W_ih.T).astype(bf)
    whhT = np.ascontiguousarray(W_hh.T).astype(bf)
    bias2 = (b_ih + b_hh).reshape(1, H).astype(bf)
    data_flat = data.reshape(N)  # t-major

    in_maps = []
    for c in range(NC):
        sl = slice(c * PSH, (c + 1) * PSH)
        samp = np.empty((128, 80), dtype=np.int32)
        for s in range(NS):
            for pt in range(8):
                samp[:, s * 8 + pt] = samples[s, c * PSH + pt * 128: c * PSH + (pt + 1) * 128]
        in_maps.append({
            "emb": emb_W,
            "wihT": wihT,
            "whhT": whhT,
            "bias2": bias2,
            "wx_idx": data_flat[sl].reshape(PSH, 1).astype(np.int32),
            "ps_idx": np.arange(c * VSH, (c + 1) * VSH, dtype=np.int32).reshape(VSH, 1),
            "samp_idx": samp,
            "prev_idx": np.arange(c * PSH, (c + 1) * PSH, dtype=np.int32).reshape(8, 128).T.copy(),
        })

    res = run_bass_kernel_spmd(nc, in_maps, core_ids=list(range(NC)))
    _CACHE["last_res"] = res
    pos = float(res.results[0]["pos_out"].ravel()[0])
    neg = sum(float(r["neg_out"].ravel()[0]) for r in res.results)
    return np.float32(pos + neg)
